# revision 20
# baseline (speedup 1.0000x reference)
"""Trainium2 Bass kernel for nn_GATON (2-layer bipartite GAT over 200k edges).

Strategy (8 NeuronCores, SPMD):
  - Big input matmul h_seq = x_seq @ Ws^T sharded over seq rows (1/8 of the
    320MB x_seq read per core).
  - Edges sharded by DESTINATION node range per conv direction, sorted by dst
    on the host. Each core owns a dst shard -> segment softmax stats and
    scatter-add are core-local (no all-reduce); only the gather-source tables
    (hs / attention scalars) are all-gathered.
  - Per dst-tile (128 dst nodes) aggregation via one-hot matmul: for each
    128-edge tile, S[e,d] = (dst_local[e]==d) built with is_equal vs iota,
    msg m[e,:] = exp(lrelu(a_s+a_d))[e,h] * hs[src[e],h,:], and PE computes
    psum[d,:] += S^T @ m (and denominators += S^T @ ex) accumulating across
    edge tiles in PSUM.
  - Per-edge hs rows + attention scalars fetched with dma_gather (SWDGE);
    src-side and dst-side scalars live in one combined table so each dst-tile
    needs a single scalar gather.
"""
import os
from contextlib import ExitStack

import numpy as np
import ml_dtypes

import concourse.bass as bass
import concourse.mybir as mybir
import concourse.tile as tile
from concourse import bacc
from concourse.bass_utils import run_bass_kernel_spmd
from concourse.masks import make_identity

F32 = mybir.dt.float32
BF16 = mybir.dt.bfloat16
I16 = mybir.dt.int16
AOT = mybir.AluOpType
AFT = mybir.ActivationFunctionType

NITEM, NSEQ, WED, D, H, OUT, E, HID, NC = 10000, 8000, 300, 128, 4, 64, 200000, 512, 8
SH_I, SH_S = NITEM // NC, NSEQ // NC          # 1250, 1000
NT_I, NT_S = 10, 8                            # dst tiles per shard (ceil/128)
KSEQ = 79                                     # ceil(10000/128)
KSEQ_PAD = KSEQ * 128                         # 10112
CHUNK = 8                                     # edge-tiles per hs gather chunk
METS = 28                                     # max edge-tiles per dst-tile
PAD_I, PAD_S = 1280, 1024                     # padded dst-shard sizes

BNP = ml_dtypes.bfloat16


def _wrap16(a):
    """[..., n] int -> dma_gather idx layout [128, n//16] int16 (16-wrap,
    replicated 8x down partitions)."""
    n = a.shape[-1]
    w = a.reshape(-1, n // 16, 16).transpose(0, 2, 1).astype(np.int16)
    return np.tile(w, (1, 8, 1))


def _fold_att(w, att):
    """w [H*C, K], att [1, H, C] -> v [K, H] with (x @ w.T * att).sum(-1) == x @ v."""
    h, c = att.shape[1], att.shape[2]
    return np.einsum("hck,hc->kh", w.reshape(h, c, -1), att[0]).astype(np.float32)


def _prep_edges(src_all, dst_all, shard, ntiles, dst_off):
    """Sort/pad per-core dst-sharded edge lists.

    Returns (n_et, src16 [NC,128,TE//16], sc16 [NC,128,2*TE//16],
    edloc [NC,128,TE//128] bf16). sc16 holds, per dst-tile, the src index
    block followed by the (local dst + dst_off) index block, for the combined
    scalar / conv2 tables."""
    counts = np.zeros((NC, ntiles), np.int64)
    percore = []
    for c in range(NC):
        sel = (dst_all // shard) == c
        s = src_all[sel]
        dl = dst_all[sel] - c * shard
        order = np.argsort(dl, kind="stable")
        s, dl = s[order], dl[order]
        percore.append((s, dl))
        counts[c] = np.bincount(dl // 128, minlength=ntiles)
    n_et = np.maximum(1, -(-counts.max(0) // 128))
    assert n_et.max() <= METS, n_et.max()
    TE = int(n_et.sum()) * 128
    src_idx = np.zeros((NC, TE), np.int64)
    sc_idx = np.full((NC, 2 * TE), dst_off, np.int64)
    edloc = np.full((NC, TE), -1.0, np.float32)
    for c in range(NC):
        s, dl = percore[c]
        off_in = 0
        off_out = 0
        for t in range(ntiles):
            cnt = int(counts[c, t])
            ets = int(n_et[t])
            sl = slice(off_out, off_out + cnt)
            src_idx[c, sl] = s[off_in:off_in + cnt]
            edloc[c, sl] = (dl[off_in:off_in + cnt] - t * 128).astype(np.float32)
            so = 2 * off_out
            sc_idx[c, so:so + cnt] = s[off_in:off_in + cnt]
            sc_idx[c, so + ets * 128:so + ets * 128 + cnt] = \
                dl[off_in:off_in + cnt] + dst_off
            off_in += cnt
            off_out += ets * 128
    edloc_w = np.ascontiguousarray(
        edloc.reshape(NC, TE // 128, 128).transpose(0, 2, 1)).astype(BNP)
    return n_et, _wrap16(src_idx), _wrap16(sc_idx), edloc_w


def _chunks(n):
    out = []
    s = 0
    while s < n:
        out.append((s, min(CHUNK, n - s)))
        s += CHUNK
    return out


def _build_program(n_et_si, n_et_is):
    nc = bacc.Bacc("TRN2", target_bir_lowering=False, debug=False,
                   num_devices=NC)
    TEsi, TEis = int(n_et_si.sum()) * 128, int(n_et_is.sum()) * 128
    RG = [list(range(NC))]

    def dram_in(name, shape, dt=F32):
        return nc.dram_tensor(name, list(shape), dt, kind="ExternalInput")

    xseqT = dram_in("xseqT", [KSEQ_PAD, SH_S])
    xitemT = dram_in("xitemT", [384, PAD_I])
    wsT = dram_in("wsT", [KSEQ_PAD, 128])
    wiT = dram_in("wiT", [384, 128])
    c1si_wT = dram_in("c1si_wT", [128, 512])
    c1is_wT = dram_in("c1is_wT", [128, 512])
    cseq1 = dram_in("cseq1", [128, 8])
    citem1 = dram_in("citem1", [128, 8])
    citem2 = dram_in("citem2", [512, 66])
    cseq2 = dram_in("cseq2", [512, 66])
    iota_in = dram_in("iota", [128, 128], BF16)
    si_src = dram_in("si_src", [128, TEsi // 16], I16)
    si_sc = dram_in("si_sc", [128, 2 * TEsi // 16], I16)
    si_edloc = dram_in("si_edloc", [128, TEsi // 128], BF16)
    is_src = dram_in("is_src", [128, TEis // 16], I16)
    is_sc = dram_in("is_sc", [128, 2 * TEis // 16], I16)
    is_edloc = dram_in("is_edloc", [128, TEis // 128], BF16)

    out_item = nc.dram_tensor("out_item", [SH_I, OUT], F32, kind="ExternalOutput")
    out_seq = nc.dram_tensor("out_seq", [SH_S, OUT], F32, kind="ExternalOutput")

    # internal DRAM. sc tables: [a_s full | a_d local shard] rows of 64 f32.
    # tb2 tables: [hs3|a_s3 full (128 bf16) | a_d local] rows of 128 bf16.
    hs1_sh = nc.dram_tensor("hs1_sh", [SH_S, HID], BF16)
    hs2_sh = nc.dram_tensor("hs2_sh", [SH_I, HID], BF16)
    hs1 = nc.dram_tensor("hs1", [NSEQ, HID], BF16, addr_space="Shared")
    hs2 = nc.dram_tensor("hs2", [NITEM, HID], BF16, addr_space="Shared")
    ap1s_sh = nc.dram_tensor("ap1s_sh", [SH_S, 64], F32)
    ap1i_sh = nc.dram_tensor("ap1i_sh", [SH_I, 64], F32)
    sc_si = nc.dram_tensor("sc_si", [NSEQ + PAD_I, 64], F32, addr_space="Shared")
    sc_is = nc.dram_tensor("sc_is", [NITEM + PAD_S, 64], F32, addr_space="Shared")
    hs3p_sh = nc.dram_tensor("hs3p_sh", [SH_S, 128], BF16)
    hs4p_sh = nc.dram_tensor("hs4p_sh", [SH_I, 128], BF16)
    tb2_si = nc.dram_tensor("tb2_si", [NSEQ + PAD_I, 128], BF16,
                            addr_space="Shared")
    tb2_is = nc.dram_tensor("tb2_is", [NITEM + PAD_S, 128], BF16,
                            addr_space="Shared")

    with tile.TileContext(nc) as tc, ExitStack() as ctx:
        pers = ctx.enter_context(tc.tile_pool(name="pers", bufs=1))

        _regs = {}
        def nreg(v):
            if v not in _regs:
                _regs[v] = nc.gpsimd.to_reg(v)
            return _regs[v]

        # PE hw-decode prefers a single un-synced dep; funnel extra cross-
        # engine deps through tiny dummy matmuls.
        peps = ctx.enter_context(tc.tile_pool(name="peps", bufs=1, space="PSUM"))
        pe_dummy = peps.tile([1, 1], F32, space="PSUM", tag="dummy")
        def pe_touch(*aps):
            for ap in aps:
                nc.tensor.matmul(pe_dummy[:1, :1], lhsT=ap, rhs=ap,
                                 start=True, stop=True, skip_group_check=True)

        iota_sb = pers.tile([128, 128], BF16, tag="iota")
        nc.sync.dma_start(iota_sb[:], iota_in[:])
        ident = pers.tile([128, 128], F32, tag="ident")
        make_identity(nc, ident[:])
        w1si = pers.tile([128, 512], F32, tag="w1si")
        nc.sync.dma_start(w1si[:], c1si_wT[:])
        w1is = pers.tile([128, 512], F32, tag="w1is")
        nc.sync.dma_start(w1is[:], c1is_wT[:])
        cs1 = pers.tile([128, 8], F32, tag="cs1")
        nc.sync.dma_start(cs1[:], cseq1[:])
        ci1 = pers.tile([128, 8], F32, tag="ci1")
        nc.sync.dma_start(ci1[:], citem1[:])
        ci2 = pers.tile([128, 4, 66], F32, tag="ci2")
        nc.sync.dma_start(ci2[:], citem2[:].rearrange("(t k) n -> k t n", k=128))
        cs2 = pers.tile([128, 4, 66], F32, tag="cs2")
        nc.sync.dma_start(cs2[:], cseq2[:].rearrange("(t k) n -> k t n", k=128))

        earr = {}
        for name, (srcd, scd, edlocd, TE) in {
            "si": (si_src, si_sc, si_edloc, TEsi),
            "is": (is_src, is_sc, is_edloc, TEis),
        }.items():
            st = pers.tile([128, TE // 16], I16, tag=f"{name}_src")
            nc.sync.dma_start(st[:], srcd[:])
            sct = pers.tile([128, 2 * TE // 16], I16, tag=f"{name}_sc")
            nc.sync.dma_start(sct[:], scd[:])
            et = pers.tile([128, TE // 128], BF16, tag=f"{name}_edloc")
            nc.sync.dma_start(et[:], edlocd[:])
            earr[name] = (st, sct, et)

        h_itemT = pers.tile([128, PAD_I], F32, tag="h_itemT")
        h_seqT = pers.tile([128, PAD_S], F32, tag="h_seqT")
        h2iT = pers.tile([128, 4, PAD_I], F32, tag="h2iT")
        h2sT = pers.tile([128, 4, PAD_S], F32, tag="h2sT")

        # ---------------- P1: h_itemT = Wi @ x_item^T (shard) ----------------
        with tc.tile_pool(name="p1", bufs=1) as p1, \
             tc.tile_pool(name="p1ps", bufs=2, space="PSUM") as p1ps:
            xit = p1.tile([128, 3, PAD_I], F32)
            nc.sync.dma_start(xit[:], xitemT[:].rearrange("(t k) n -> k t n", k=128))
            wit = p1.tile([128, 3, 128], F32)
            nc.sync.dma_start(wit[:], wiT[:].rearrange("(t k) m -> k t m", k=128))
            pe_touch(wit[:, 0, 0:1], xit[:, 0, 0:1])
            for ntl in range(3):
                n0 = ntl * 512
                nn = min(PAD_I, n0 + 512) - n0
                ps = p1ps.tile([128, 512], F32, space="PSUM")
                for k in range(3):
                    nc.tensor.matmul(ps[:, :nn], lhsT=wit[:, k, :],
                                     rhs=xit[:, k, n0:n0 + nn],
                                     start=(k == 0), stop=(k == 2))
                nc.vector.tensor_copy(h_itemT[:, n0:n0 + nn], ps[:, :nn])

        # ------------- P4 (moved early): item-side layer-1 tables ------------
        # runs before the big x_seq matmul so the hs2/ap1i allgathers overlap
        # with P2's DMA streaming.
        def l1_tables(hT, ntiles, nvalid, wt, combo, hs_sh, hs_full, ap_sh,
                      sc_full, sc_off, p3, p3ps):
            pe_touch(wt[:, 0:1], combo[:, 0:1])
            for t in range(ntiles):
                n0 = t * 128
                nv = min(nvalid, n0 + 128) - n0
                psA = p3ps.tile([128, 512], F32, space="PSUM", tag="psA")
                nc.tensor.matmul(psA[:], lhsT=hT[:, n0:n0 + 128], rhs=wt[:],
                                 start=True, stop=True)
                psB = p3ps.tile([128, 8], F32, space="PSUM", tag="psB")
                nc.tensor.matmul(psB[:], lhsT=hT[:, n0:n0 + 128], rhs=combo[:],
                                 start=True, stop=True)
                tA = p3.tile([128, HID], BF16, tag="tA")
                nc.vector.tensor_copy(tA[:], psA[:])
                nc.sync.dma_start(hs_sh[n0:n0 + nv, :], tA[:nv, :])
                tB = p3.tile([128, 64], F32, tag="tB")
                nc.vector.memset(tB[:, 4:64], 0.0)
                nc.vector.tensor_copy(tB[:, 0:4], psB[:, 0:4])
                nc.sync.dma_start(ap_sh[n0:n0 + nv, :], tB[:nv, :])
                tC = p3.tile([128, 64], F32, tag="tC")
                nc.vector.memset(tC[:, 4:64], 0.0)
                nc.vector.tensor_copy(tC[:, 0:4], psB[:, 4:8])
                nc.sync.dma_start(sc_full[sc_off + n0:sc_off + n0 + 128, :],
                                  tC[:, :])
            nc.gpsimd.collective_compute(
                "AllGather", AOT.bypass, ins=[hs_sh[:]], outs=[hs_full[:]],
                replica_groups=RG)

        with tc.tile_pool(name="p4", bufs=3) as p4, \
             tc.tile_pool(name="p4ps", bufs=3, space="PSUM") as p4ps:
            # item side: combo=[a_s2 | a_d1]; a_d1 rows land in sc_si
            l1_tables(h_itemT, NT_I, SH_I, w1is, ci1, hs2_sh, hs2, ap1i_sh,
                      sc_si, NSEQ, p4, p4ps)
        nc.gpsimd.collective_compute(
            "AllGather", AOT.bypass, ins=[ap1i_sh[:]], outs=[sc_is[0:NITEM, :]],
            replica_groups=RG)

        # ---------------- P2: h_seqT = Ws @ x_seq^T (shard) ------------------
        nc.vector.memset(h_seqT[:, SH_S:PAD_S], 0.0)
        with tc.tile_pool(name="p2w", bufs=1) as p2w, \
             tc.tile_pool(name="p2x", bufs=3) as p2x, \
             tc.tile_pool(name="p2ps", bufs=1, space="PSUM") as p2ps:
            wst = p2w.tile([128, KSEQ, 128], F32)
            nc.sync.dma_start(wst[:], wsT[:].rearrange("(t k) m -> k t m", k=128))
            pe_touch(wst[:, 0, 0:1])
            pe_touch(h_itemT[:, 0:1])
            ps0 = p2ps.tile([128, 512], F32, space="PSUM", tag="ps0")
            ps1 = p2ps.tile([128, 512], F32, space="PSUM", tag="ps1")
            for kb in range(0, KSEQ, 2):
                kn = min(2, KSEQ - kb)
                xt = p2x.tile([128, 2, SH_S], F32)
                nc.sync.dma_start(
                    xt[:, :kn, :],
                    xseqT[kb * 128:(kb + kn) * 128, :]
                        .rearrange("(t k) n -> k t n", k=128))
                for tt in range(kn):
                    k = kb + tt
                    nc.tensor.matmul(ps0[:, :512], lhsT=wst[:, k, :],
                                     rhs=xt[:, tt, 0:512],
                                     start=(k == 0), stop=(k == KSEQ - 1),
                                     skip_group_check=True)
                    nc.tensor.matmul(ps1[:, :488], lhsT=wst[:, k, :],
                                     rhs=xt[:, tt, 512:1000],
                                     start=(k == 0), stop=(k == KSEQ - 1),
                                     skip_group_check=True)
            nc.vector.tensor_copy(h_seqT[:, 0:512], ps0[:, :512])
            nc.vector.tensor_copy(h_seqT[:, 512:1000], ps1[:, :488])

        # ------------- P3: seq-side layer-1 tables ---------------------------
        with tc.tile_pool(name="p3", bufs=3) as p3, \
             tc.tile_pool(name="p3ps", bufs=3, space="PSUM") as p3ps:
            # seq side: combo=[a_s1 | a_d2]; a_d2 rows land in sc_is
            l1_tables(h_seqT, NT_S, SH_S, w1si, cs1, hs1_sh, hs1, ap1s_sh,
                      sc_is, NITEM, p3, p3ps)
        nc.gpsimd.collective_compute(
            "AllGather", AOT.bypass, ins=[ap1s_sh[:]], outs=[sc_si[0:NSEQ, :]],
            replica_groups=RG)

        # ------------- conv1 edge pass (per direction) -----------------------
        def conv1_pass(dname, n_et, hs_full, sc_full, h2T, pools):
            src_sb, sc_sb, edloc_sb = earr[dname]
            gpool, scpool, mspool, eppool, psM, psS, psT = pools
            eoff = 0
            for t in range(len(n_et)):
                psumM = psM.tile([128, HID], F32, space="PSUM", tag="m")
                psumS = psS.tile([128, 4], F32, space="PSUM", tag="s")
                ets = int(n_et[t])
                # one combined scalar gather per dst-tile: [a_s | a_d]
                scg = scpool.tile([128, 2 * METS, 64], F32, tag="scg")
                nc.gpsimd.dma_gather(
                    out_ap=scg[:, :2 * ets, :], in_ap=sc_full[:],
                    idxs_ap=sc_sb[:, eoff * 16:(eoff + ets) * 16],
                    num_idxs=2 * ets * 128, num_idxs_reg=nreg(2 * ets * 128),
                    elem_size=64, single_packet=False)
                al = mspool.tile([128, METS, 4], F32, tag="al")
                nc.vector.tensor_tensor(out=al[:, :ets, :],
                                        in0=scg[:, :ets, 0:4],
                                        in1=scg[:, ets:2 * ets, 0:4], op=AOT.add)
                al2 = mspool.tile([128, METS, 4], F32, tag="al2")
                nc.vector.tensor_scalar_mul(al2[:, :ets, :], al[:, :ets, :], 0.2)
                nc.vector.tensor_tensor(out=al2[:, :ets, :], in0=al[:, :ets, :],
                                        in1=al2[:, :ets, :], op=AOT.max)
                exf = mspool.tile([128, METS, 4], F32, tag="exf")
                nc.scalar.activation(exf[:, :ets, :], al2[:, :ets, :], AFT.Exp)
                ex = mspool.tile([128, METS, 4], BF16, tag="ex")
                nc.vector.tensor_copy(ex[:, :ets, :], exf[:, :ets, :])
                for ci_, (cs, cn) in enumerate(_chunks(ets)):
                    e0 = eoff + cs
                    g = gpool.tile([128, CHUNK, HID], BF16, tag="g")
                    nc.gpsimd.dma_gather(
                        out_ap=g[:, :cn, :], in_ap=hs_full[:],
                        idxs_ap=src_sb[:, e0 * 8:(e0 + cn) * 8],
                        num_idxs=cn * 128, num_idxs_reg=nreg(cn * 128),
                        elem_size=HID)
                    S = mspool.tile([128, CHUNK, 128], BF16, tag="S")
                    nc.vector.tensor_tensor(
                        out=S[:, :cn, :],
                        in0=edloc_sb[:, e0:e0 + cn].unsqueeze(2)
                            .to_broadcast([128, cn, 128]),
                        in1=iota_sb[:].unsqueeze(1).to_broadcast([128, cn, 128]),
                        op=AOT.is_equal)
                    m = mspool.tile([128, CHUNK, HID], BF16, tag="mm")
                    if ci_ % 2 == 0:
                        nc.vector.tensor_tensor(
                            out=m[:, :cn, :].rearrange("p t (h c) -> p t h c", h=4),
                            in0=g[:, :cn, :].rearrange("p t (h c) -> p t h c", h=4),
                            in1=ex[:, cs:cs + cn, :].unsqueeze(3)
                                .to_broadcast([128, cn, 4, 128]),
                            op=AOT.mult)
                    else:
                        # ACT path: per (tile, head) copy with per-partition scale
                        for et_ in range(cn):
                            for hh in range(4):
                                nc.scalar.activation(
                                    m[:, et_, hh * 128:(hh + 1) * 128],
                                    g[:, et_, hh * 128:(hh + 1) * 128],
                                    AFT.Copy,
                                    scale=exf[:, cs + et_, hh:hh + 1])
                    for et in range(cn):
                        first = (ci_ == 0 and et == 0)
                        last = (cs + et == ets - 1)
                        nc.tensor.matmul(psumM[:], lhsT=S[:, et, :],
                                         rhs=m[:, et, :], start=first,
                                         stop=last, skip_group_check=True)
                        nc.tensor.matmul(psumS[:], lhsT=S[:, et, :],
                                         rhs=ex[:, cs + et, :], start=first,
                                         stop=last, skip_group_check=True)
                # epilogue: divide by denom, elu, transpose into h2T
                rec = eppool.tile([128, 4], F32, tag="rec")
                nc.vector.tensor_scalar_add(rec[:], psumS[:], 1e-16)
                nc.vector.reciprocal(rec[:], rec[:])
                h2 = eppool.tile([128, HID], F32, tag="h2")
                nc.vector.tensor_tensor(
                    out=h2[:].rearrange("p (h c) -> p h c", h=4),
                    in0=psumM[:].rearrange("p (h c) -> p h c", h=4),
                    in1=rec[:].unsqueeze(2).to_broadcast([128, 4, 128]),
                    op=AOT.mult)
                e1 = eppool.tile([128, HID], F32, tag="e1")
                nc.scalar.activation(e1[:], h2[:], AFT.Exp)
                e2 = eppool.tile([128, HID], F32, tag="e2")
                nc.scalar.activation(e2[:], e1[:], AFT.Relu, bias=1.0, scale=-1.0)
                e3 = eppool.tile([128, HID], F32, tag="e3")
                nc.vector.tensor_scalar_max(e3[:], h2[:], 0.0)
                nc.vector.tensor_tensor(out=e3[:], in0=e3[:], in1=e2[:],
                                        op=AOT.subtract)
                for cb in range(4):
                    pt = psT.tile([128, 128], F32, space="PSUM", tag="t")
                    nc.tensor.transpose(pt[:], e3[:, cb * 128:(cb + 1) * 128],
                                        ident[:])
                    nc.vector.tensor_copy(h2T[:, cb, t * 128:(t + 1) * 128], pt[:])
                eoff += ets

        # ------------- conv2 transform tables --------------------------------
        def l2_tables(h2T, ntiles, nvalid, combo, hsp_sh, tb_full, tb_off,
                      p7, p7ps):
            pe_touch(combo[:, 0, 0:1])
            for t in range(ntiles):
                n0 = t * 128
                nv = min(nvalid, n0 + 128) - n0
                ps = p7ps.tile([128, 66], F32, space="PSUM", tag="ps")
                for k in range(4):
                    nc.tensor.matmul(ps[:], lhsT=h2T[:, k, n0:n0 + 128],
                                     rhs=combo[:, k, :],
                                     start=(k == 0), stop=(k == 3))
                tp = p7.tile([128, 128], BF16, tag="tp")
                nc.vector.tensor_copy(tp[:, 0:65], ps[:, 0:65])
                nc.vector.memset(tp[:, 65:128], 0.0)
                hi = p7.tile([128, 2], F32, tag="hi")
                nc.vector.tensor_copy(hi[:], tp[:, 64:66])  # upcast hi parts
                lo = p7.tile([128, 2], F32, tag="lo")
                nc.vector.tensor_tensor(out=lo[:], in0=ps[:, 64:66], in1=hi[:],
                                        op=AOT.subtract)
                nc.vector.tensor_copy(tp[:, 65:66], lo[:, 0:1])  # a_s lo
                nc.sync.dma_start(hsp_sh[n0:n0 + nv, :], tp[:nv, :])
                ta = p7.tile([128, 128], BF16, tag="ta")
                nc.vector.memset(ta[:, 2:128], 0.0)
                nc.vector.tensor_copy(ta[:, 0:1], ps[:, 65:66])  # a_d hi
                adhi = p7.tile([128, 1], F32, tag="adhi")
                nc.vector.tensor_copy(adhi[:], ta[:, 0:1])
                adlo = p7.tile([128, 1], F32, tag="adlo")
                nc.vector.tensor_tensor(out=adlo[:], in0=ps[:, 65:66],
                                        in1=adhi[:], op=AOT.subtract)
                nc.vector.tensor_copy(ta[:, 1:2], adlo[:])  # a_d lo
                nc.sync.dma_start(tb_full[tb_off + n0:tb_off + n0 + 128, :],
                                  ta[:, :])


        pe_touch(ident[:, 0:1], iota_sb[:, 0:1])
        with tc.tile_pool(name="g", bufs=3) as gpool, \
             tc.tile_pool(name="sc", bufs=2) as scpool, \
             tc.tile_pool(name="ms", bufs=3) as mspool, \
             tc.tile_pool(name="ep", bufs=2) as eppool, \
             tc.tile_pool(name="p7", bufs=3) as p7, \
             tc.tile_pool(name="psM", bufs=2, space="PSUM") as psM, \
             tc.tile_pool(name="psS", bufs=2, space="PSUM") as psS, \
             tc.tile_pool(name="psT", bufs=1, space="PSUM") as psT, \
             tc.tile_pool(name="p7ps", bufs=2, space="PSUM") as p7ps:
            pools = (gpool, scpool, mspool, eppool, psM, psS, psT)
            conv1_pass("si", n_et_si, hs1, sc_si, h2iT, pools)
            # item-side conv2 tables depend only on h2iT -> overlap conv1_is
            l2_tables(h2iT, NT_I, SH_I, ci2, hs4p_sh, tb2_si, NSEQ, p7, p7ps)
            nc.gpsimd.collective_compute(
                "AllGather", AOT.bypass, ins=[hs4p_sh[:]],
                outs=[tb2_is[0:NITEM, :]], replica_groups=RG)
            conv1_pass("is", n_et_is, hs2, sc_is, h2sT, pools)
            l2_tables(h2sT, NT_S, SH_S, cs2, hs3p_sh, tb2_is, NITEM, p7,
                      p7ps)
            nc.gpsimd.collective_compute(
                "AllGather", AOT.bypass, ins=[hs3p_sh[:]],
                outs=[tb2_si[0:NSEQ, :]], replica_groups=RG)

        # ------------- conv2 edge pass (per direction) -----------------------
        def conv2_pass(dname, n_et, tb_full, out_dram, nvalid, pools):
            src_sb, sc_sb, edloc_sb = earr[dname]
            gpool, mspool, eppool, psM, psS = pools
            eoff = 0
            for t in range(len(n_et)):
                psumM = psM.tile([128, OUT], F32, space="PSUM", tag="m")
                psumS = psS.tile([128, 1], F32, space="PSUM", tag="s")
                ets = int(n_et[t])
                # one combined gather: [hs3|a_s3 rows, then a_d rows]
                g = gpool.tile([128, 2 * METS, 128], BF16, tag="g")
                nc.gpsimd.dma_gather(
                    out_ap=g[:, :2 * ets, :], in_ap=tb_full[:],
                    idxs_ap=sc_sb[:, eoff * 16:(eoff + ets) * 16],
                    num_idxs=2 * ets * 128, num_idxs_reg=nreg(2 * ets * 128),
                    elem_size=128, single_packet=False)
                al = mspool.tile([128, METS, 1], F32, tag="al")
                alo = mspool.tile([128, METS, 1], F32, tag="alo")
                nc.vector.tensor_tensor(out=al[:, :ets, :],
                                        in0=g[:, :ets, 64:65],
                                        in1=g[:, ets:2 * ets, 0:1], op=AOT.add)
                nc.vector.tensor_tensor(out=alo[:, :ets, :],
                                        in0=g[:, :ets, 65:66],
                                        in1=g[:, ets:2 * ets, 1:2], op=AOT.add)
                nc.vector.tensor_tensor(out=al[:, :ets, :],
                                        in0=al[:, :ets, :],
                                        in1=alo[:, :ets, :], op=AOT.add)
                al2 = mspool.tile([128, METS, 1], F32, tag="al2")
                nc.vector.tensor_scalar_mul(al2[:, :ets, :], al[:, :ets, :], 0.2)
                nc.vector.tensor_tensor(out=al2[:, :ets, :], in0=al[:, :ets, :],
                                        in1=al2[:, :ets, :], op=AOT.max)
                ex = mspool.tile([128, METS, 1], BF16, tag="ex")
                nc.scalar.activation(ex[:, :ets, :], al2[:, :ets, :], AFT.Exp)
                S = mspool.tile([128, METS, 128], BF16, tag="S")
                nc.vector.tensor_tensor(
                    out=S[:, :ets, :],
                    in0=edloc_sb[:, eoff:eoff + ets].unsqueeze(2)
                        .to_broadcast([128, ets, 128]),
                    in1=iota_sb[:].unsqueeze(1).to_broadcast([128, ets, 128]),
                    op=AOT.is_equal)
                m = mspool.tile([128, METS, OUT], BF16, tag="mm")
                nc.vector.tensor_tensor(
                    out=m[:, :ets, :], in0=g[:, :ets, 0:64],
                    in1=ex[:, :ets, :].to_broadcast([128, ets, OUT]),
                    op=AOT.mult)
                for et in range(ets):
                    first = (et == 0)
                    last = (et == ets - 1)
                    nc.tensor.matmul(psumM[:], lhsT=S[:, et, :], rhs=m[:, et, :],
                                     start=first, stop=last,
                                     skip_group_check=True)
                    nc.tensor.matmul(psumS[:], lhsT=S[:, et, :],
                                     rhs=ex[:, et, :], start=first, stop=last,
                                     skip_group_check=True)
                # epilogue: divide, softmax over 64, store
                nv = min(nvalid, (t + 1) * 128) - t * 128
                rec = eppool.tile([128, 1], F32, tag="rec")
                nc.vector.tensor_scalar_add(rec[:], psumS[:], 1e-16)
                nc.vector.reciprocal(rec[:], rec[:])
                o = eppool.tile([128, OUT], F32, tag="o")
                nc.vector.tensor_scalar_mul(o[:], psumM[:], rec[:, 0:1])
                mx = eppool.tile([128, 1], F32, tag="mx")
                nc.vector.tensor_reduce(mx[:], o[:], axis=mybir.AxisListType.X,
                                        op=AOT.max)
                nc.vector.tensor_scalar_mul(mx[:], mx[:], -1.0)
                sm = eppool.tile([128, 1], F32, tag="sm")
                eo = eppool.tile([128, OUT], F32, tag="eo")
                nc.scalar.activation(eo[:], o[:], AFT.Exp, bias=mx[:, 0:1],
                                     accum_out=sm[:])
                nc.vector.reciprocal(sm[:], sm[:])
                nc.vector.tensor_scalar_mul(eo[:], eo[:], sm[:, 0:1])
                nc.sync.dma_start(out_dram[t * 128:t * 128 + nv, :], eo[:nv, :])
                eoff += ets

        with tc.tile_pool(name="g2", bufs=3) as gpool2, \
             tc.tile_pool(name="ms2", bufs=3) as mspool2, \
             tc.tile_pool(name="ep2", bufs=2) as eppool2, \
             tc.tile_pool(name="psM2", bufs=2, space="PSUM") as psM2, \
             tc.tile_pool(name="psS2", bufs=2, space="PSUM") as psS2:
            pools2 = (gpool2, mspool2, eppool2, psM2, psS2)
            conv2_pass("si", n_et_si, tb2_si, out_item, SH_I, pools2)
            conv2_pass("is", n_et_is, tb2_is, out_seq, SH_S, pools2)

    nc.compile()
    return nc


_CACHE = {}


def kernel(x_item, x_seq, edge_index, params):
    x_item = np.asarray(x_item, dtype=np.float32)
    x_seq = np.asarray(x_seq, dtype=np.float32)
    edge_index = np.asarray(edge_index)
    p = {k: {kk: np.asarray(vv, dtype=np.float32) for kk, vv in v.items()}
         if isinstance(v, dict) else np.asarray(v, dtype=np.float32)
         for k, v in params.items()}
    for key in ("Wi_b", "Ws_b"):
        assert np.all(p[key] == 0.0), f"nonzero {key} unsupported"
    for ck in ("c1_si", "c1_is", "c2_si", "c2_is"):
        assert np.all(p[ck]["bias"] == 0.0), f"nonzero {ck}.bias unsupported"

    e_seq = edge_index[0].astype(np.int64)
    e_item = edge_index[1].astype(np.int64)
    loop = np.arange(NSEQ, dtype=np.int64)
    src_si = np.concatenate([e_seq, loop])
    dst_si = np.concatenate([e_item, loop])
    src_is = np.concatenate([e_item, loop])
    dst_is = np.concatenate([e_seq, loop])

    n_et_si, si_src, si_sc, si_edloc = _prep_edges(src_si, dst_si, SH_I, NT_I,
                                                   NSEQ)
    n_et_is, is_src, is_sc, is_edloc = _prep_edges(src_is, dst_is, SH_S, NT_S,
                                                   NITEM)

    key = (tuple(n_et_si), tuple(n_et_is))
    if key not in _CACHE:
        _CACHE[key] = _build_program(n_et_si, n_et_is)
    nc = _CACHE[key]

    wsT = np.zeros((KSEQ_PAD, 128), np.float32)
    wsT[:NITEM] = p["Ws_w"].T
    wiT = np.zeros((384, 128), np.float32)
    wiT[:WED] = p["Wi_w"].T
    c1si, c1is, c2si, c2is = p["c1_si"], p["c1_is"], p["c2_si"], p["c2_is"]
    cseq1 = np.concatenate(
        [_fold_att(c1si["w_src"], c1si["att_src"]),
         _fold_att(c1is["w_dst"], c1is["att_dst"])], axis=1)      # [128, 8]
    citem1 = np.concatenate(
        [_fold_att(c1is["w_src"], c1is["att_src"]),
         _fold_att(c1si["w_dst"], c1si["att_dst"])], axis=1)      # [128, 8]
    citem2 = np.concatenate(
        [c2is["w_src"].T.astype(np.float32),
         _fold_att(c2is["w_src"], c2is["att_src"]),
         _fold_att(c2si["w_dst"], c2si["att_dst"])], axis=1)      # [512, 66]
    cseq2 = np.concatenate(
        [c2si["w_src"].T.astype(np.float32),
         _fold_att(c2si["w_src"], c2si["att_src"]),
         _fold_att(c2is["w_dst"], c2is["att_dst"])], axis=1)      # [512, 66]
    iota = np.tile(np.arange(128, dtype=np.float32), (128, 1)).astype(BNP)

    xseqT = np.zeros((KSEQ_PAD, NSEQ), np.float32)
    xseqT[:NITEM] = x_seq.T
    xitemT = np.zeros((384, NITEM), np.float32)
    xitemT[:WED] = x_item.T

    shared = dict(wsT=wsT, wiT=wiT, c1si_wT=c1si["w_src"].T.copy(),
                  c1is_wT=c1is["w_src"].T.copy(), cseq1=cseq1, citem1=citem1,
                  citem2=citem2, cseq2=cseq2, iota=iota)
    in_maps = []
    for c in range(NC):
        xi = np.zeros((384, PAD_I), np.float32)
        xi[:, :SH_I] = xitemT[:, c * SH_I:(c + 1) * SH_I]
        m = dict(shared)
        m.update(
            xseqT=np.ascontiguousarray(xseqT[:, c * SH_S:(c + 1) * SH_S]),
            xitemT=xi,
            si_src=si_src[c], si_sc=si_sc[c], si_edloc=si_edloc[c],
            is_src=is_src[c], is_sc=is_sc[c], is_edloc=is_edloc[c],
        )
        in_maps.append(m)

    kernel.last_in_maps = in_maps
    trace = os.environ.get("GATON_TRACE", "0") == "1"
    res = run_bass_kernel_spmd(nc, in_maps, list(range(NC)), trace=trace)
    if res.exec_time_ns is not None:
        kernel.last_exec_ns = res.exec_time_ns
    kernel.last_results = res
    h_item3 = np.concatenate([res.results[c]["out_item"] for c in range(NC)], axis=0)
    h_seq3 = np.concatenate([res.results[c]["out_seq"] for c in range(NC)], axis=0)
    return (h_item3, h_seq3)


# revision 21
# speedup vs baseline: 1.1443x; 1.1443x over previous
"""Trainium2 Bass kernel for nn_GATON (2-layer bipartite GAT over 200k edges).

Strategy (8 NeuronCores, SPMD):
  - Big input matmul h_seq = x_seq @ Ws^T sharded over seq rows (1/8 of the
    320MB x_seq read per core).
  - Edges sharded by DESTINATION node range per conv direction, sorted by dst
    on the host. Each core owns a dst shard -> segment softmax stats and
    scatter-add are core-local (no all-reduce); only the gather-source tables
    (hs / attention scalars) are all-gathered.
  - Per dst-tile (128 dst nodes) aggregation via one-hot matmul: for each
    128-edge tile, S[e,d] = (dst_local[e]==d) built with is_equal vs iota,
    msg m[e,:] = exp(lrelu(a_s+a_d))[e,h] * hs[src[e],h,:], and PE computes
    psum[d,:] += S^T @ m (and denominators += S^T @ ex) accumulating across
    edge tiles in PSUM.
  - Per-edge hs rows + attention scalars fetched with dma_gather (SWDGE);
    src-side and dst-side scalars live in one combined table so each dst-tile
    needs a single scalar gather.
"""
import os
from contextlib import ExitStack

import numpy as np
import ml_dtypes

import concourse.bass as bass
import concourse.mybir as mybir
import concourse.tile as tile
from concourse import bacc
from concourse.bass_utils import run_bass_kernel_spmd
from concourse.masks import make_identity

F32 = mybir.dt.float32
BF16 = mybir.dt.bfloat16
I16 = mybir.dt.int16
AOT = mybir.AluOpType
AFT = mybir.ActivationFunctionType

NITEM, NSEQ, WED, D, H, OUT, E, HID, NC = 10000, 8000, 300, 128, 4, 64, 200000, 512, 8
SH_I, SH_S = NITEM // NC, NSEQ // NC          # 1250, 1000
NT_I, NT_S = 10, 8                            # dst tiles per shard (ceil/128)
KSEQ = 79                                     # ceil(10000/128)
KSEQ_PAD = KSEQ * 128                         # 10112
CHUNK = 8                                     # edge-tiles per hs gather chunk
METS = 28                                     # max edge-tiles per dst-tile
PAD_I, PAD_S = 1280, 1024                     # padded dst-shard sizes

BNP = ml_dtypes.bfloat16


def _wrap16(a):
    """[..., n] int -> dma_gather idx layout [128, n//16] int16 (16-wrap,
    replicated 8x down partitions)."""
    n = a.shape[-1]
    w = a.reshape(-1, n // 16, 16).transpose(0, 2, 1).astype(np.int16)
    return np.tile(w, (1, 8, 1))


def _fold_att(w, att):
    """w [H*C, K], att [1, H, C] -> v [K, H] with (x @ w.T * att).sum(-1) == x @ v."""
    h, c = att.shape[1], att.shape[2]
    return np.einsum("hck,hc->kh", w.reshape(h, c, -1), att[0]).astype(np.float32)


def _prep_edges(src_all, dst_all, shard, ntiles, dst_off):
    """Sort/pad per-core dst-sharded edge lists.

    Returns (n_et, src16 [NC,128,TE//16], sc16 [NC,128,2*TE//16],
    edloc [NC,128,TE//128] bf16). sc16 holds, per dst-tile, the src index
    block followed by the (local dst + dst_off) index block, for the combined
    scalar / conv2 tables."""
    counts = np.zeros((NC, ntiles), np.int64)
    percore = []
    for c in range(NC):
        sel = (dst_all // shard) == c
        s = src_all[sel]
        dl = dst_all[sel] - c * shard
        order = np.argsort(dl, kind="stable")
        s, dl = s[order], dl[order]
        percore.append((s, dl))
        counts[c] = np.bincount(dl // 128, minlength=ntiles)
    n_et = np.maximum(1, -(-counts.max(0) // 128))
    assert n_et.max() <= METS, n_et.max()
    TE = int(n_et.sum()) * 128
    src_idx = np.zeros((NC, TE), np.int64)
    sc_idx = np.full((NC, 2 * TE), dst_off, np.int64)
    edloc = np.full((NC, TE), -1.0, np.float32)
    for c in range(NC):
        s, dl = percore[c]
        off_in = 0
        off_out = 0
        for t in range(ntiles):
            cnt = int(counts[c, t])
            ets = int(n_et[t])
            sl = slice(off_out, off_out + cnt)
            src_idx[c, sl] = s[off_in:off_in + cnt]
            edloc[c, sl] = (dl[off_in:off_in + cnt] - t * 128).astype(np.float32)
            so = 2 * off_out
            sc_idx[c, so:so + cnt] = s[off_in:off_in + cnt]
            sc_idx[c, so + ets * 128:so + ets * 128 + cnt] = \
                dl[off_in:off_in + cnt] + dst_off
            off_in += cnt
            off_out += ets * 128
    edloc_w = np.ascontiguousarray(
        edloc.reshape(NC, TE // 128, 128).transpose(0, 2, 1)).astype(BNP)
    return n_et, _wrap16(src_idx), _wrap16(sc_idx), edloc_w


def _chunks(n):
    out = []
    s = 0
    while s < n:
        out.append((s, min(CHUNK, n - s)))
        s += CHUNK
    return out


def _build_program(n_et_si, n_et_is):
    nc = bacc.Bacc("TRN2", target_bir_lowering=False, debug=False,
                   num_devices=NC)
    TEsi, TEis = int(n_et_si.sum()) * 128, int(n_et_is.sum()) * 128
    RG = [list(range(NC))]

    def dram_in(name, shape, dt=F32):
        return nc.dram_tensor(name, list(shape), dt, kind="ExternalInput")

    xseqT = dram_in("xseqT", [KSEQ_PAD, SH_S])
    xitemT = dram_in("xitemT", [384, PAD_I])
    wsT = dram_in("wsT", [KSEQ_PAD, 128])
    wiT = dram_in("wiT", [384, 128])
    c1si_wT = dram_in("c1si_wT", [128, 512])
    c1is_wT = dram_in("c1is_wT", [128, 512])
    cseq1 = dram_in("cseq1", [128, 8])
    citem1 = dram_in("citem1", [128, 8])
    citem2 = dram_in("citem2", [512, 66])
    cseq2 = dram_in("cseq2", [512, 66])
    iota_in = dram_in("iota", [128, 128], BF16)
    si_src = dram_in("si_src", [128, TEsi // 16], I16)
    si_sc = dram_in("si_sc", [128, 2 * TEsi // 16], I16)
    si_edloc = dram_in("si_edloc", [128, TEsi // 128], BF16)
    is_src = dram_in("is_src", [128, TEis // 16], I16)
    is_sc = dram_in("is_sc", [128, 2 * TEis // 16], I16)
    is_edloc = dram_in("is_edloc", [128, TEis // 128], BF16)

    out_item = nc.dram_tensor("out_item", [SH_I, OUT], F32, kind="ExternalOutput")
    out_seq = nc.dram_tensor("out_seq", [SH_S, OUT], F32, kind="ExternalOutput")

    # internal DRAM. sc tables: [a_s full | a_d local shard] rows of 64 f32.
    # tb2 tables: [hs3|a_s3 full (128 bf16) | a_d local] rows of 128 bf16.
    hs1_sh = nc.dram_tensor("hs1_sh", [SH_S, HID], BF16)
    hs2_sh = nc.dram_tensor("hs2_sh", [SH_I, HID], BF16)
    hs1 = nc.dram_tensor("hs1", [NSEQ, HID], BF16, addr_space="Shared")
    hs2 = nc.dram_tensor("hs2", [NITEM, HID], BF16, addr_space="Shared")
    ap1s_sh = nc.dram_tensor("ap1s_sh", [SH_S, 64], F32)
    ap1i_sh = nc.dram_tensor("ap1i_sh", [SH_I, 64], F32)
    sc_si = nc.dram_tensor("sc_si", [NSEQ + PAD_I, 64], F32, addr_space="Shared")
    sc_is = nc.dram_tensor("sc_is", [NITEM + PAD_S, 64], F32, addr_space="Shared")
    hs3p_sh = nc.dram_tensor("hs3p_sh", [SH_S, 128], BF16)
    hs4p_sh = nc.dram_tensor("hs4p_sh", [SH_I, 128], BF16)
    tb2_si = nc.dram_tensor("tb2_si", [NSEQ + PAD_I, 128], BF16,
                            addr_space="Shared")
    tb2_is = nc.dram_tensor("tb2_is", [NITEM + PAD_S, 128], BF16,
                            addr_space="Shared")

    with tile.TileContext(nc) as tc, ExitStack() as ctx:
        pers = ctx.enter_context(tc.tile_pool(name="pers", bufs=1))

        _regs = {}
        def nreg(v):
            if v not in _regs:
                _regs[v] = nc.gpsimd.to_reg(v)
            return _regs[v]

        # PE hw-decode prefers a single un-synced dep; funnel extra cross-
        # engine deps through tiny dummy matmuls.
        peps = ctx.enter_context(tc.tile_pool(name="peps", bufs=1, space="PSUM"))
        pe_dummy = peps.tile([1, 1], F32, space="PSUM", tag="dummy")
        def pe_touch(*aps):
            for ap in aps:
                nc.tensor.matmul(pe_dummy[:1, :1], lhsT=ap, rhs=ap,
                                 start=True, stop=True, skip_group_check=True)

        iota_sb = pers.tile([128, 128], BF16, tag="iota")
        nc.sync.dma_start(iota_sb[:], iota_in[:])
        ident = pers.tile([128, 128], F32, tag="ident")
        make_identity(nc, ident[:])
        w1si = pers.tile([128, 512], F32, tag="w1si")
        nc.sync.dma_start(w1si[:], c1si_wT[:])
        w1is = pers.tile([128, 512], F32, tag="w1is")
        nc.sync.dma_start(w1is[:], c1is_wT[:])
        cs1 = pers.tile([128, 8], F32, tag="cs1")
        nc.sync.dma_start(cs1[:], cseq1[:])
        ci1 = pers.tile([128, 8], F32, tag="ci1")
        nc.sync.dma_start(ci1[:], citem1[:])
        ci2 = pers.tile([128, 4, 66], F32, tag="ci2")
        nc.sync.dma_start(ci2[:], citem2[:].rearrange("(t k) n -> k t n", k=128))
        cs2 = pers.tile([128, 4, 66], F32, tag="cs2")
        nc.sync.dma_start(cs2[:], cseq2[:].rearrange("(t k) n -> k t n", k=128))

        earr = {}
        for name, (srcd, scd, edlocd, TE) in {
            "si": (si_src, si_sc, si_edloc, TEsi),
            "is": (is_src, is_sc, is_edloc, TEis),
        }.items():
            st = pers.tile([128, TE // 16], I16, tag=f"{name}_src")
            nc.sync.dma_start(st[:], srcd[:])
            sct = pers.tile([128, 2 * TE // 16], I16, tag=f"{name}_sc")
            nc.sync.dma_start(sct[:], scd[:])
            et = pers.tile([128, TE // 128], BF16, tag=f"{name}_edloc")
            nc.sync.dma_start(et[:], edlocd[:])
            earr[name] = (st, sct, et)

        h_itemT = pers.tile([128, PAD_I], F32, tag="h_itemT")
        h_seqT = pers.tile([128, PAD_S], F32, tag="h_seqT")
        h2iT = pers.tile([128, 4, PAD_I], F32, tag="h2iT")
        h2sT = pers.tile([128, 4, PAD_S], F32, tag="h2sT")

        # ---------------- P1: h_itemT = Wi @ x_item^T (shard) ----------------
        with tc.tile_pool(name="p1", bufs=1) as p1, \
             tc.tile_pool(name="p1ps", bufs=2, space="PSUM") as p1ps:
            xit = p1.tile([128, 3, PAD_I], F32)
            nc.sync.dma_start(xit[:], xitemT[:].rearrange("(t k) n -> k t n", k=128))
            wit = p1.tile([128, 3, 128], F32)
            nc.sync.dma_start(wit[:], wiT[:].rearrange("(t k) m -> k t m", k=128))
            pe_touch(wit[:, 0, 0:1], xit[:, 0, 0:1])
            for ntl in range(3):
                n0 = ntl * 512
                nn = min(PAD_I, n0 + 512) - n0
                ps = p1ps.tile([128, 512], F32, space="PSUM")
                for k in range(3):
                    nc.tensor.matmul(ps[:, :nn], lhsT=wit[:, k, :],
                                     rhs=xit[:, k, n0:n0 + nn],
                                     start=(k == 0), stop=(k == 2))
                nc.vector.tensor_copy(h_itemT[:, n0:n0 + nn], ps[:, :nn])

        # ------------- P4 (moved early): item-side layer-1 tables ------------
        # runs before the big x_seq matmul so the hs2/ap1i allgathers overlap
        # with P2's DMA streaming.
        def l1_tables(hT, ntiles, nvalid, wt, combo, hs_sh, hs_full, ap_sh,
                      sc_full, sc_off, p3, p3ps):
            pe_touch(wt[:, 0:1], combo[:, 0:1])
            for t in range(ntiles):
                n0 = t * 128
                nv = min(nvalid, n0 + 128) - n0
                psA = p3ps.tile([128, 512], F32, space="PSUM", tag="psA")
                nc.tensor.matmul(psA[:], lhsT=hT[:, n0:n0 + 128], rhs=wt[:],
                                 start=True, stop=True)
                psB = p3ps.tile([128, 8], F32, space="PSUM", tag="psB")
                nc.tensor.matmul(psB[:], lhsT=hT[:, n0:n0 + 128], rhs=combo[:],
                                 start=True, stop=True)
                tA = p3.tile([128, HID], BF16, tag="tA")
                nc.vector.tensor_copy(tA[:], psA[:])
                nc.sync.dma_start(hs_sh[n0:n0 + nv, :], tA[:nv, :])
                tB = p3.tile([128, 64], F32, tag="tB")
                nc.vector.memset(tB[:, 4:64], 0.0)
                nc.vector.tensor_copy(tB[:, 0:4], psB[:, 0:4])
                nc.sync.dma_start(ap_sh[n0:n0 + nv, :], tB[:nv, :])
                tC = p3.tile([128, 64], F32, tag="tC")
                nc.vector.memset(tC[:, 4:64], 0.0)
                nc.vector.tensor_copy(tC[:, 0:4], psB[:, 4:8])
                nc.sync.dma_start(sc_full[sc_off + n0:sc_off + n0 + 128, :],
                                  tC[:, :])
            nc.gpsimd.collective_compute(
                "AllGather", AOT.bypass, ins=[hs_sh[:]], outs=[hs_full[:]],
                replica_groups=RG)

        with tc.tile_pool(name="p4", bufs=3) as p4, \
             tc.tile_pool(name="p4ps", bufs=3, space="PSUM") as p4ps:
            # item side: combo=[a_s2 | a_d1]; a_d1 rows land in sc_si
            l1_tables(h_itemT, NT_I, SH_I, w1is, ci1, hs2_sh, hs2, ap1i_sh,
                      sc_si, NSEQ, p4, p4ps)
        nc.gpsimd.collective_compute(
            "AllGather", AOT.bypass, ins=[ap1i_sh[:]], outs=[sc_is[0:NITEM, :]],
            replica_groups=RG)

        # ---------------- P2: h_seqT = Ws @ x_seq^T (shard) ------------------
        nc.vector.memset(h_seqT[:, SH_S:PAD_S], 0.0)
        with tc.tile_pool(name="p2w", bufs=1) as p2w, \
             tc.tile_pool(name="p2x", bufs=3) as p2x, \
             tc.tile_pool(name="p2ps", bufs=1, space="PSUM") as p2ps:
            wst = p2w.tile([128, KSEQ, 128], F32)
            nc.sync.dma_start(wst[:], wsT[:].rearrange("(t k) m -> k t m", k=128))
            pe_touch(wst[:, 0, 0:1])
            pe_touch(h_itemT[:, 0:1])
            ps0 = p2ps.tile([128, 512], F32, space="PSUM", tag="ps0")
            ps1 = p2ps.tile([128, 512], F32, space="PSUM", tag="ps1")
            for kb in range(0, KSEQ, 2):
                kn = min(2, KSEQ - kb)
                xt = p2x.tile([128, 2, SH_S], F32)
                nc.sync.dma_start(
                    xt[:, :kn, :],
                    xseqT[kb * 128:(kb + kn) * 128, :]
                        .rearrange("(t k) n -> k t n", k=128))
                for tt in range(kn):
                    k = kb + tt
                    nc.tensor.matmul(ps0[:, :512], lhsT=wst[:, k, :],
                                     rhs=xt[:, tt, 0:512],
                                     start=(k == 0), stop=(k == KSEQ - 1),
                                     skip_group_check=True)
                    nc.tensor.matmul(ps1[:, :488], lhsT=wst[:, k, :],
                                     rhs=xt[:, tt, 512:1000],
                                     start=(k == 0), stop=(k == KSEQ - 1),
                                     skip_group_check=True)
            nc.vector.tensor_copy(h_seqT[:, 0:512], ps0[:, :512])
            nc.vector.tensor_copy(h_seqT[:, 512:1000], ps1[:, :488])

        # ------------- P3: seq-side layer-1 tables ---------------------------
        with tc.tile_pool(name="p3", bufs=3) as p3, \
             tc.tile_pool(name="p3ps", bufs=3, space="PSUM") as p3ps:
            # seq side: combo=[a_s1 | a_d2]; a_d2 rows land in sc_is
            l1_tables(h_seqT, NT_S, SH_S, w1si, cs1, hs1_sh, hs1, ap1s_sh,
                      sc_is, NITEM, p3, p3ps)
        nc.gpsimd.collective_compute(
            "AllGather", AOT.bypass, ins=[ap1s_sh[:]], outs=[sc_si[0:NSEQ, :]],
            replica_groups=RG)

        # ------------- conv1 edge pass (per direction) -----------------------
        def conv1_pass(dname, n_et, hs_full, sc_full, h2T, pools):
            src_sb, sc_sb, edloc_sb = earr[dname]
            gpool, scpool, mspool, eppool, psM, psS, psT = pools
            eoff = 0
            for t in range(len(n_et)):
                psumM = psM.tile([128, HID], F32, space="PSUM", tag="m")
                psumS = psS.tile([128, 4], F32, space="PSUM", tag="s")
                ets = int(n_et[t])
                # one combined scalar gather per dst-tile: [a_s | a_d]
                scg = scpool.tile([128, 2 * METS, 64], F32, tag="scg")
                nc.gpsimd.dma_gather(
                    out_ap=scg[:, :2 * ets, :], in_ap=sc_full[:],
                    idxs_ap=sc_sb[:, eoff * 16:(eoff + ets) * 16],
                    num_idxs=2 * ets * 128, num_idxs_reg=nreg(2 * ets * 128),
                    elem_size=64, single_packet=False)
                al = mspool.tile([128, METS, 4], F32, tag="al")
                nc.vector.tensor_tensor(out=al[:, :ets, :],
                                        in0=scg[:, :ets, 0:4],
                                        in1=scg[:, ets:2 * ets, 0:4], op=AOT.add)
                al2 = mspool.tile([128, METS, 4], F32, tag="al2")
                nc.vector.tensor_scalar_mul(al2[:, :ets, :], al[:, :ets, :], 0.2)
                nc.vector.tensor_tensor(out=al2[:, :ets, :], in0=al[:, :ets, :],
                                        in1=al2[:, :ets, :], op=AOT.max)
                exf = mspool.tile([128, METS, 4], F32, tag="exf")
                nc.scalar.activation(exf[:, :ets, :], al2[:, :ets, :], AFT.Exp)
                ex = mspool.tile([128, METS, 4], BF16, tag="ex")
                nc.vector.tensor_copy(ex[:, :ets, :], exf[:, :ets, :])
                for ci_, (cs, cn) in enumerate(_chunks(ets)):
                    e0 = eoff + cs
                    g = gpool.tile([128, CHUNK, HID], BF16, tag="g")
                    nc.gpsimd.dma_gather(
                        out_ap=g[:, :cn, :], in_ap=hs_full[:],
                        idxs_ap=src_sb[:, e0 * 8:(e0 + cn) * 8],
                        num_idxs=cn * 128, num_idxs_reg=nreg(cn * 128),
                        elem_size=HID)
                    S = mspool.tile([128, CHUNK, 128], BF16, tag="S")
                    nc.vector.tensor_tensor(
                        out=S[:, :cn, :],
                        in0=edloc_sb[:, e0:e0 + cn].unsqueeze(2)
                            .to_broadcast([128, cn, 128]),
                        in1=iota_sb[:].unsqueeze(1).to_broadcast([128, cn, 128]),
                        op=AOT.is_equal)
                    m = mspool.tile([128, CHUNK, HID], BF16, tag="mm")
                    if ci_ % 2 == 0:
                        nc.vector.tensor_tensor(
                            out=m[:, :cn, :].rearrange("p t (h c) -> p t h c", h=4),
                            in0=g[:, :cn, :].rearrange("p t (h c) -> p t h c", h=4),
                            in1=ex[:, cs:cs + cn, :].unsqueeze(3)
                                .to_broadcast([128, cn, 4, 128]),
                            op=AOT.mult)
                    else:
                        # ACT path: per (tile, head) copy with per-partition scale
                        for et_ in range(cn):
                            for hh in range(4):
                                nc.scalar.activation(
                                    m[:, et_, hh * 128:(hh + 1) * 128],
                                    g[:, et_, hh * 128:(hh + 1) * 128],
                                    AFT.Copy,
                                    scale=exf[:, cs + et_, hh:hh + 1])
                    for et in range(cn):
                        first = (ci_ == 0 and et == 0)
                        last = (cs + et == ets - 1)
                        nc.tensor.matmul(psumM[:], lhsT=S[:, et, :],
                                         rhs=m[:, et, :], start=first,
                                         stop=last, skip_group_check=True)
                        nc.tensor.matmul(psumS[:], lhsT=S[:, et, :],
                                         rhs=ex[:, cs + et, :], start=first,
                                         stop=last, skip_group_check=True)
                # epilogue: divide by denom, elu, transpose into h2T
                rec = eppool.tile([128, 4], F32, tag="rec")
                nc.vector.tensor_scalar_add(rec[:], psumS[:], 1e-16)
                nc.vector.reciprocal(rec[:], rec[:])
                h2 = eppool.tile([128, HID], F32, tag="h2")
                nc.vector.tensor_tensor(
                    out=h2[:].rearrange("p (h c) -> p h c", h=4),
                    in0=psumM[:].rearrange("p (h c) -> p h c", h=4),
                    in1=rec[:].unsqueeze(2).to_broadcast([128, 4, 128]),
                    op=AOT.mult)
                e1 = eppool.tile([128, HID], F32, tag="e1")
                nc.scalar.activation(e1[:], h2[:], AFT.Exp)
                e2 = eppool.tile([128, HID], F32, tag="e2")
                nc.scalar.activation(e2[:], e1[:], AFT.Relu, bias=1.0, scale=-1.0)
                e3 = eppool.tile([128, HID], F32, tag="e3")
                nc.vector.tensor_scalar_max(e3[:], h2[:], 0.0)
                nc.vector.tensor_tensor(out=e3[:], in0=e3[:], in1=e2[:],
                                        op=AOT.subtract)
                for cb in range(4):
                    pt = psT.tile([128, 128], F32, space="PSUM", tag="t")
                    nc.tensor.transpose(pt[:], e3[:, cb * 128:(cb + 1) * 128],
                                        ident[:])
                    nc.vector.tensor_copy(h2T[:, cb, t * 128:(t + 1) * 128], pt[:])
                eoff += ets

        # ------------- conv2 transform tables --------------------------------
        def l2_tables(h2T, ntiles, nvalid, combo, hsp_sh, tb_full, tb_off,
                      p7, p7ps):
            pe_touch(combo[:, 0, 0:1])
            for t in range(ntiles):
                n0 = t * 128
                nv = min(nvalid, n0 + 128) - n0
                ps = p7ps.tile([128, 66], F32, space="PSUM", tag="ps")
                for k in range(4):
                    nc.tensor.matmul(ps[:], lhsT=h2T[:, k, n0:n0 + 128],
                                     rhs=combo[:, k, :],
                                     start=(k == 0), stop=(k == 3))
                tp = p7.tile([128, 128], BF16, tag="tp")
                nc.vector.tensor_copy(tp[:, 0:65], ps[:, 0:65])
                nc.vector.memset(tp[:, 65:128], 0.0)
                hi = p7.tile([128, 2], F32, tag="hi")
                nc.vector.tensor_copy(hi[:], tp[:, 64:66])  # upcast hi parts
                lo = p7.tile([128, 2], F32, tag="lo")
                nc.vector.tensor_tensor(out=lo[:], in0=ps[:, 64:66], in1=hi[:],
                                        op=AOT.subtract)
                nc.vector.tensor_copy(tp[:, 65:66], lo[:, 0:1])  # a_s lo
                nc.sync.dma_start(hsp_sh[n0:n0 + nv, :], tp[:nv, :])
                ta = p7.tile([128, 128], BF16, tag="ta")
                nc.vector.memset(ta[:, 2:128], 0.0)
                nc.vector.tensor_copy(ta[:, 0:1], ps[:, 65:66])  # a_d hi
                adhi = p7.tile([128, 1], F32, tag="adhi")
                nc.vector.tensor_copy(adhi[:], ta[:, 0:1])
                adlo = p7.tile([128, 1], F32, tag="adlo")
                nc.vector.tensor_tensor(out=adlo[:], in0=ps[:, 65:66],
                                        in1=adhi[:], op=AOT.subtract)
                nc.vector.tensor_copy(ta[:, 1:2], adlo[:])  # a_d lo
                nc.sync.dma_start(tb_full[tb_off + n0:tb_off + n0 + 128, :],
                                  ta[:, :])


        pe_touch(ident[:, 0:1], iota_sb[:, 0:1])
        with tc.tile_pool(name="g", bufs=3) as gpool, \
             tc.tile_pool(name="sc", bufs=2) as scpool, \
             tc.tile_pool(name="ms", bufs=3) as mspool, \
             tc.tile_pool(name="ep", bufs=2) as eppool, \
             tc.tile_pool(name="p7", bufs=3) as p7, \
             tc.tile_pool(name="psM", bufs=2, space="PSUM") as psM, \
             tc.tile_pool(name="psS", bufs=2, space="PSUM") as psS, \
             tc.tile_pool(name="psT", bufs=1, space="PSUM") as psT, \
             tc.tile_pool(name="p7ps", bufs=2, space="PSUM") as p7ps:
            pools = (gpool, scpool, mspool, eppool, psM, psS, psT)
            # "is" first: its tables (hs2/sc_is) complete before the seq-side
            # allgathers, so its gathers fill the DMA hole after P2.
            conv1_pass("is", n_et_is, hs2, sc_is, h2sT, pools)
            l2_tables(h2sT, NT_S, SH_S, cs2, hs3p_sh, tb2_is, NITEM, p7,
                      p7ps)
            nc.gpsimd.collective_compute(
                "AllGather", AOT.bypass, ins=[hs3p_sh[:]],
                outs=[tb2_si[0:NSEQ, :]], replica_groups=RG)
            conv1_pass("si", n_et_si, hs1, sc_si, h2iT, pools)
            l2_tables(h2iT, NT_I, SH_I, ci2, hs4p_sh, tb2_si, NSEQ, p7, p7ps)
            nc.gpsimd.collective_compute(
                "AllGather", AOT.bypass, ins=[hs4p_sh[:]],
                outs=[tb2_is[0:NITEM, :]], replica_groups=RG)

        # ------------- conv2 edge pass (per direction) -----------------------
        def conv2_pass(dname, n_et, tb_full, out_dram, nvalid, pools):
            src_sb, sc_sb, edloc_sb = earr[dname]
            gpool, mspool, eppool, psM, psS = pools
            eoff = 0
            for t in range(len(n_et)):
                psumM = psM.tile([128, OUT], F32, space="PSUM", tag="m")
                psumS = psS.tile([128, 1], F32, space="PSUM", tag="s")
                ets = int(n_et[t])
                # one combined gather: [hs3|a_s3 rows, then a_d rows]
                g = gpool.tile([128, 2 * METS, 128], BF16, tag="g")
                nc.gpsimd.dma_gather(
                    out_ap=g[:, :2 * ets, :], in_ap=tb_full[:],
                    idxs_ap=sc_sb[:, eoff * 16:(eoff + ets) * 16],
                    num_idxs=2 * ets * 128, num_idxs_reg=nreg(2 * ets * 128),
                    elem_size=128, single_packet=False)
                al = mspool.tile([128, METS, 1], F32, tag="al")
                alo = mspool.tile([128, METS, 1], F32, tag="alo")
                nc.vector.tensor_tensor(out=al[:, :ets, :],
                                        in0=g[:, :ets, 64:65],
                                        in1=g[:, ets:2 * ets, 0:1], op=AOT.add)
                nc.vector.tensor_tensor(out=alo[:, :ets, :],
                                        in0=g[:, :ets, 65:66],
                                        in1=g[:, ets:2 * ets, 1:2], op=AOT.add)
                nc.vector.tensor_tensor(out=al[:, :ets, :],
                                        in0=al[:, :ets, :],
                                        in1=alo[:, :ets, :], op=AOT.add)
                al2 = mspool.tile([128, METS, 1], F32, tag="al2")
                nc.vector.tensor_scalar_mul(al2[:, :ets, :], al[:, :ets, :], 0.2)
                nc.vector.tensor_tensor(out=al2[:, :ets, :], in0=al[:, :ets, :],
                                        in1=al2[:, :ets, :], op=AOT.max)
                ex = mspool.tile([128, METS, 1], BF16, tag="ex")
                nc.scalar.activation(ex[:, :ets, :], al2[:, :ets, :], AFT.Exp)
                S = mspool.tile([128, METS, 128], BF16, tag="S")
                nc.vector.tensor_tensor(
                    out=S[:, :ets, :],
                    in0=edloc_sb[:, eoff:eoff + ets].unsqueeze(2)
                        .to_broadcast([128, ets, 128]),
                    in1=iota_sb[:].unsqueeze(1).to_broadcast([128, ets, 128]),
                    op=AOT.is_equal)
                m = mspool.tile([128, METS, OUT], BF16, tag="mm")
                nc.vector.tensor_tensor(
                    out=m[:, :ets, :], in0=g[:, :ets, 0:64],
                    in1=ex[:, :ets, :].to_broadcast([128, ets, OUT]),
                    op=AOT.mult)
                for et in range(ets):
                    first = (et == 0)
                    last = (et == ets - 1)
                    nc.tensor.matmul(psumM[:], lhsT=S[:, et, :], rhs=m[:, et, :],
                                     start=first, stop=last,
                                     skip_group_check=True)
                    nc.tensor.matmul(psumS[:], lhsT=S[:, et, :],
                                     rhs=ex[:, et, :], start=first, stop=last,
                                     skip_group_check=True)
                # epilogue: divide, softmax over 64, store
                nv = min(nvalid, (t + 1) * 128) - t * 128
                rec = eppool.tile([128, 1], F32, tag="rec")
                nc.vector.tensor_scalar_add(rec[:], psumS[:], 1e-16)
                nc.vector.reciprocal(rec[:], rec[:])
                o = eppool.tile([128, OUT], F32, tag="o")
                nc.vector.tensor_scalar_mul(o[:], psumM[:], rec[:, 0:1])
                mx = eppool.tile([128, 1], F32, tag="mx")
                nc.vector.tensor_reduce(mx[:], o[:], axis=mybir.AxisListType.X,
                                        op=AOT.max)
                nc.vector.tensor_scalar_mul(mx[:], mx[:], -1.0)
                sm = eppool.tile([128, 1], F32, tag="sm")
                eo = eppool.tile([128, OUT], F32, tag="eo")
                nc.scalar.activation(eo[:], o[:], AFT.Exp, bias=mx[:, 0:1],
                                     accum_out=sm[:])
                nc.vector.reciprocal(sm[:], sm[:])
                nc.vector.tensor_scalar_mul(eo[:], eo[:], sm[:, 0:1])
                nc.sync.dma_start(out_dram[t * 128:t * 128 + nv, :], eo[:nv, :])
                eoff += ets

        with tc.tile_pool(name="g2", bufs=3) as gpool2, \
             tc.tile_pool(name="ms2", bufs=3) as mspool2, \
             tc.tile_pool(name="ep2", bufs=2) as eppool2, \
             tc.tile_pool(name="psM2", bufs=2, space="PSUM") as psM2, \
             tc.tile_pool(name="psS2", bufs=2, space="PSUM") as psS2:
            pools2 = (gpool2, mspool2, eppool2, psM2, psS2)
            conv2_pass("si", n_et_si, tb2_si, out_item, SH_I, pools2)
            conv2_pass("is", n_et_is, tb2_is, out_seq, SH_S, pools2)

    nc.compile()
    return nc


_CACHE = {}


def kernel(x_item, x_seq, edge_index, params):
    x_item = np.asarray(x_item, dtype=np.float32)
    x_seq = np.asarray(x_seq, dtype=np.float32)
    edge_index = np.asarray(edge_index)
    p = {k: {kk: np.asarray(vv, dtype=np.float32) for kk, vv in v.items()}
         if isinstance(v, dict) else np.asarray(v, dtype=np.float32)
         for k, v in params.items()}
    for key in ("Wi_b", "Ws_b"):
        assert np.all(p[key] == 0.0), f"nonzero {key} unsupported"
    for ck in ("c1_si", "c1_is", "c2_si", "c2_is"):
        assert np.all(p[ck]["bias"] == 0.0), f"nonzero {ck}.bias unsupported"

    e_seq = edge_index[0].astype(np.int64)
    e_item = edge_index[1].astype(np.int64)
    loop = np.arange(NSEQ, dtype=np.int64)
    src_si = np.concatenate([e_seq, loop])
    dst_si = np.concatenate([e_item, loop])
    src_is = np.concatenate([e_item, loop])
    dst_is = np.concatenate([e_seq, loop])

    n_et_si, si_src, si_sc, si_edloc = _prep_edges(src_si, dst_si, SH_I, NT_I,
                                                   NSEQ)
    n_et_is, is_src, is_sc, is_edloc = _prep_edges(src_is, dst_is, SH_S, NT_S,
                                                   NITEM)

    key = (tuple(n_et_si), tuple(n_et_is))
    if key not in _CACHE:
        _CACHE[key] = _build_program(n_et_si, n_et_is)
    nc = _CACHE[key]

    wsT = np.zeros((KSEQ_PAD, 128), np.float32)
    wsT[:NITEM] = p["Ws_w"].T
    wiT = np.zeros((384, 128), np.float32)
    wiT[:WED] = p["Wi_w"].T
    c1si, c1is, c2si, c2is = p["c1_si"], p["c1_is"], p["c2_si"], p["c2_is"]
    cseq1 = np.concatenate(
        [_fold_att(c1si["w_src"], c1si["att_src"]),
         _fold_att(c1is["w_dst"], c1is["att_dst"])], axis=1)      # [128, 8]
    citem1 = np.concatenate(
        [_fold_att(c1is["w_src"], c1is["att_src"]),
         _fold_att(c1si["w_dst"], c1si["att_dst"])], axis=1)      # [128, 8]
    citem2 = np.concatenate(
        [c2is["w_src"].T.astype(np.float32),
         _fold_att(c2is["w_src"], c2is["att_src"]),
         _fold_att(c2si["w_dst"], c2si["att_dst"])], axis=1)      # [512, 66]
    cseq2 = np.concatenate(
        [c2si["w_src"].T.astype(np.float32),
         _fold_att(c2si["w_src"], c2si["att_src"]),
         _fold_att(c2is["w_dst"], c2is["att_dst"])], axis=1)      # [512, 66]
    iota = np.tile(np.arange(128, dtype=np.float32), (128, 1)).astype(BNP)

    xseqT = np.zeros((KSEQ_PAD, NSEQ), np.float32)
    xseqT[:NITEM] = x_seq.T
    xitemT = np.zeros((384, NITEM), np.float32)
    xitemT[:WED] = x_item.T

    shared = dict(wsT=wsT, wiT=wiT, c1si_wT=c1si["w_src"].T.copy(),
                  c1is_wT=c1is["w_src"].T.copy(), cseq1=cseq1, citem1=citem1,
                  citem2=citem2, cseq2=cseq2, iota=iota)
    in_maps = []
    for c in range(NC):
        xi = np.zeros((384, PAD_I), np.float32)
        xi[:, :SH_I] = xitemT[:, c * SH_I:(c + 1) * SH_I]
        m = dict(shared)
        m.update(
            xseqT=np.ascontiguousarray(xseqT[:, c * SH_S:(c + 1) * SH_S]),
            xitemT=xi,
            si_src=si_src[c], si_sc=si_sc[c], si_edloc=si_edloc[c],
            is_src=is_src[c], is_sc=is_sc[c], is_edloc=is_edloc[c],
        )
        in_maps.append(m)

    kernel.last_in_maps = in_maps
    trace = os.environ.get("GATON_TRACE", "0") == "1"
    res = run_bass_kernel_spmd(nc, in_maps, list(range(NC)), trace=trace)
    if res.exec_time_ns is not None:
        kernel.last_exec_ns = res.exec_time_ns
    kernel.last_results = res
    h_item3 = np.concatenate([res.results[c]["out_item"] for c in range(NC)], axis=0)
    h_seq3 = np.concatenate([res.results[c]["out_seq"] for c in range(NC)], axis=0)
    return (h_item3, h_seq3)
